# revision 56
# baseline (speedup 1.0000x reference)
"""Deformable 3D conv net on 8 Trainium2 NeuronCores (Bass/Tile).

Sharding: core (b, q) = batch b in {0,1} x D-quarter q in {0..3};
each core computes out[b, :, 12q:12q+12, :, :] from a padded x slab.

Per-core algorithm (exact trilinear, 5-wide window, exact for |off|<=2;
offsets clamped to [-2,2] on device; actual max |off| ~ 1.83):
  1. PE off-conv: off[81, 48,48] per d-slice, contraction K=96
     (3 w-shift replicas x 32 channels) accumulated over 9 (kd,kh) taps.
  2. Per tap k, zeta[(dd,dh,dw), n] = prod_ax relu(1-|off_ax - dvec|)
     is built entirely on PE+ACT in log space (no DVE work): a
     selection matmul replicates the 3 off rows (bf16 hi+lo pairs) to
     15 PSUM rows, ACT computes ln(hat), a K=15 matmul sums the 3
     log-hats per delta, ACT exps into bf16 zeta.
  3. Taps grouped by kd (3 groups of 9). Per group, per channel c:
     one replicating DMA builds xr[125, HP, WP] = 125 delta-shifted
     replicas of the 5-plane padded x window for that kd; per tap:
     P = zeta * xr_window (DVE bf16, the roofline resource); PE matmul
     K=125 with stationary w_dc[o,c,k] broadcast over rows accumulates
     out[32, h, w] in PSUM across all (k, c).
  Work is software-pipelined: the next stage's zeta builds (PE/ACT)
  and the next slice's off-conv are emitted inside the current
  stage's channel loop, so DVE never waits at stage boundaries.
"""

import numpy as np
import ml_dtypes

import concourse.bass as bass
import concourse.bacc as bacc
import concourse.mybir as mybir
from concourse.tile import TileContext
from concourse.bass_utils import run_bass_kernel_spmd

B, C, O, S = 2, 32, 32, 48
KS, KV = 3, 27
PAD = 4
DP = 12                 # output D per core
DPP = DP + 2 * PAD      # 20
HP = WP = S + 2 * PAD   # 56
HWP = HP * WP           # 3136
NPAD = DPP * HWP        # 62720
NDELTA = 125

F32 = mybir.dt.float32
BF16 = mybir.dt.bfloat16
ALU = mybir.AluOpType
ACTF = mybir.ActivationFunctionType

HCHUNKS = [(0, 10), (10, 10), (20, 10), (30, 10), (40, 8)]  # h-row chunks
NS_LOOP = DP  # number of d-slices traced (reduce for simulation tests)
LAST_RESULTS = None


# ---------------------------------------------------------------- host prep
def _build_core_inputs(x, w_off, b_off, w_dc, b_dc, b, q):
    xp = np.zeros((C, DPP, HP, WP), np.float32)
    d0 = DP * q - PAD
    lo, hi = max(0, -d0), min(DPP, S - d0)
    xp[:, lo:hi, PAD:PAD + S, PAD:PAD + S] = x[b, :, d0 + lo:d0 + hi]

    # x3[32g+c, d, h, w] = xp[c, d, h, w + (g-1)]  (wrap lands in zero pad)
    x3 = np.zeros((96, DPP, HP, WP), np.float32)
    for g in range(3):
        x3[32 * g:32 * g + 32] = np.roll(xp, -(g - 1), axis=3)
    x3 = x3.reshape(96, NPAD).astype(ml_dtypes.bfloat16)

    x_bf = xp.reshape(C, NPAD).astype(ml_dtypes.bfloat16)

    # w_off9: [9*96, 81]: chunk (kd,kh), rows (kw, c), cols m = 3k + axis
    woff = w_off.reshape(KV, 3, C, KS, KS, KS)
    w_off9 = np.zeros((9, 96, 81), np.float32)
    for kd in range(3):
        for kh in range(3):
            ch = kd * 3 + kh
            for kw in range(3):
                blk = woff[:, :, :, kd, kh, kw]          # (k, ax, c)
                w_off9[ch, 32 * kw:32 * kw + 32, :] = \
                    blk.transpose(2, 0, 1).reshape(C, KV * 3)
    w_off9 = w_off9.astype(ml_dtypes.bfloat16)

    # wdc_rep: [128, KV*C*O]: rows = delta (125 used), free (k, c, o)
    wdcf = w_dc.reshape(O, C, KV)
    wdc = np.zeros((128, KV * C * O), np.float32)
    wdc[:NDELTA, :] = wdcf.transpose(2, 1, 0).reshape(KV * C * O)[None, :]
    wdc = wdc.astype(ml_dtypes.bfloat16)

    # dvec15: rows (ax, v): the delta value v-2 for each axis
    dvec15 = np.tile(np.arange(-2, 3), 3).astype(np.float32)[:, None]

    # seltap[p, 16k + 5ax + v] = 1 iff p is the off row of (tap k, axis
    # ax): rows 3k+ax, except taps 10/21 (rows crossing a 32-partition
    # boundary) which read copies placed at rows 88..90 / 93..95
    seltap = np.zeros((96, 27 * 16), ml_dtypes.bfloat16)
    for k in range(KV):
        for ax in range(3):
            row = {10: 88, 21: 93}.get(k, 3 * k) + ax
            seltap[row, 16 * k + 5 * ax:16 * k + 5 * ax + 5] = 1.0

    # logsel[(ax,v), (dd,dh,dw)] = 1 iff delta component of axis ax == v-2
    logsel = np.zeros((15, 128), ml_dtypes.bfloat16)
    for dd_ in range(5):
        for dh_ in range(5):
            for dw_ in range(5):
                d = 25 * dd_ + 5 * dh_ + dw_
                logsel[dd_, d] = 1.0
                logsel[5 + dh_, d] = 1.0
                logsel[10 + dw_, d] = 1.0

    return {
        "x3": np.ascontiguousarray(x3),
        "x_bf": np.ascontiguousarray(x_bf),
        "w_off9": np.ascontiguousarray(w_off9.transpose(1, 0, 2).reshape(96, 9 * 81)),
        "wdc_rep": np.ascontiguousarray(wdc),
        "b_off": np.ascontiguousarray(b_off.astype(np.float32).reshape(81, 1)),
        "b_dc": np.ascontiguousarray(b_dc.astype(np.float32).reshape(32, 1)),
        "dvec15": dvec15,
        "epsv": np.full((15, 1), 1e-20, np.float32),
        "seltap": seltap,
        "logsel": logsel,
    }


# ---------------------------------------------------------------- device IR
def _win_ap(dram_row_ap, offset, ap_dims):
    a = dram_row_ap.copy()
    a.ap = mybir.VecI64Pair(ap_dims)
    a.offset = offset
    return a


class _Emitter:
    """Holds the shared tiles/pools and emits the pipelined program."""

    def __init__(self, nc, pool, psp, x3_d, xbf_d, out_d, woff_s, wdc_s,
                 boff_s, bdc_s, dvec15, epsv, sel_s, logsel_s):
        self.nc = nc
        self.pool = pool
        self.psp = psp
        self.x3_d, self.xbf_d, self.out_d = x3_d, xbf_d, out_d
        self.woff_s, self.wdc_s = woff_s, wdc_s
        self.boff_s, self.bdc_s = boff_s, bdc_s
        self.dvec15, self.epsv = dvec15, epsv
        self.sel_s, self.logsel_s = sel_s, logsel_s
        self.rps = psp.tile([15, 10, S], F32, name="rps", tag="rps")
        self.zps = psp.tile([NDELTA, 10, S], F32, name="zps", tag="zps")
        self.uts = [pool.tile([15, S, S], BF16, name=f"ut{i}",
                              tag=f"ut{i}") for i in range(2)]
        self.offpair = {}
        self.accs = {}
        self.first_mm = None

    def emit_offconv_load(self, ds):
        nc, pool = self.nc, self.pool
        dpad = ds + PAD
        x3s = pool.tile([96, 3, HP, WP], BF16, name=f"x3s{ds}", tag="x3s")
        nc.gpsimd.dma_start(
            x3s.rearrange("p a h w -> p (a h w)"),
            self.x3_d[:, (dpad - 1) * HWP:(dpad + 2) * HWP])
        self._x3s = x3s
        self._off = pool.tile([81, S, S], F32, name=f"off{ds}", tag="off")

    def emit_offconv_chunk(self, ds, hc):
        nc = self.nc
        hb, hn = HCHUNKS[hc]
        ps = self.psp.tile([81, hn, S], F32, name=f"offps{ds}_{hc}",
                           tag="offps")
        for i in range(9):
            kd, kh = i // 3, i % 3
            rhs = self._x3s[:, kd, 3 + kh + hb:3 + kh + hb + hn, 4:52]
            nc.tensor.matmul(ps[:], self.woff_s[:, i * 81:(i + 1) * 81],
                             rhs, start=(i == 0), stop=(i == 8))
        # evict + bias + clamp to [-2, 2]
        nc.vector.tensor_scalar(self._off[:, hb:hb + hn, :], ps[:],
                                self.boff_s[:, :], 2.0, ALU.add, ALU.min)

    def emit_offconv_finish(self, ds):
        nc, pool = self.nc, self.pool
        off = self._off
        nc.vector.tensor_scalar(off[:], off[:], -2.0, None, ALU.max)
        # split off into bf16 hi+lo so the zeta broadcasts can use bf16
        # matmuls (fp32 rhs streams ~3x slower on the PE); rows 81..95
        # hold copies so taps 10/21 (rows crossing a 32-partition
        # boundary) can be selected from partition base 64
        off_hi = pool.tile([96, S, S], BF16, name=f"offhi{ds}", tag="offhi")
        off_lo = pool.tile([96, S, S], BF16, name=f"offlo{ds}", tag="offlo")
        tmp32 = pool.tile([81, S, S], F32, name=f"offt{ds}", tag="outp")
        nc.vector.tensor_copy(off_hi[0:81], off[:])
        nc.vector.tensor_copy(tmp32[:], off_hi[0:81])
        nc.vector.tensor_tensor(off_lo[0:81], off[:], tmp32[:], ALU.subtract)
        for t in (off_hi, off_lo):
            tf = t.rearrange("p h w -> p (h w)")
            nc.scalar.dma_start(tf[81:84, :], tf[78:81, :])
            nc.scalar.dma_start(tf[84:91, :], tf[26:33, :])
            nc.scalar.dma_start(tf[91:96, :], tf[61:66, :])
        self.offpair[ds] = (off_hi, off_lo)

    def emit_offconv(self, ds):
        self.emit_offconv_load(ds)
        for hc in range(len(HCHUNKS)):
            self.emit_offconv_chunk(ds, hc)
        self.emit_offconv_finish(ds)

    def emit_zeta(self, si, j, zetas):
        """Build zeta for tap j of stage si into a rotating z tile."""
        nc = self.nc
        ds, g = divmod(si, 3)
        k = 9 * g + j
        off_hi, off_lo = self.offpair[ds]
        z = self.pool.tile([NDELTA, S, S], BF16, name=f"z{si}_{k}",
                           tag=f"z{(9 * si + j) % 13}")
        ut = self.uts[(9 * si + j) % 2]
        base = 64 if k in (10, 21) else 32 * ((3 * k) // 32)
        lv = self.sel_s[base:base + 32, 16 * k:16 * k + 15]
        for ci, (hb, hn) in enumerate(HCHUNKS):
            nc.tensor.matmul(self.rps[:, 0:hn, :], lv,
                             off_hi[base:base + 32, hb:hb + hn, :],
                             start=True, stop=False, tile_position=(base, 0))
            nc.tensor.matmul(self.rps[:, 0:hn, :], lv,
                             off_lo[base:base + 32, hb:hb + hn, :],
                             start=False, stop=True, tile_position=(base, 0))
            # u = |dvec - off| on the scalar engine, straight from PSUM
            nc.scalar.activation(ut[:, hb:hb + hn, :], self.rps[:, 0:hn, :],
                                 ACTF.Abs, bias=self.dvec15[:, :], scale=-1.0)
        # h = relu(1 - u); L = ln(h + eps)   (both in place)
        nc.scalar.activation(ut[:], ut[:], ACTF.Relu, bias=1.0, scale=-1.0)
        nc.scalar.activation(ut[:], ut[:], ACTF.Ln, bias=self.epsv[:, :],
                             scale=1.0)
        for ci, (hb, hn) in enumerate(HCHUNKS):
            nc.tensor.matmul(self.zps[:, 0:hn, :],
                             self.logsel_s[0:15, 0:NDELTA],
                             ut[:, hb:hb + hn, :], start=True, stop=True)
            nc.scalar.activation(z[:, hb:hb + hn, :], self.zps[:, 0:hn, :],
                                 ACTF.Exp, bias=0.0, scale=1.0)
        zetas[j] = z

    def emit_stage(self, si, zetas, nstages):
        """Channel loop for stage si (slice ds, kd group g), with the
        next stage's zeta builds and next slice's off-conv interleaved."""
        nc, pool = self.nc, self.pool
        ds, g = divmod(si, 3)
        kd = g
        dpad = ds + PAD
        if g == 0:
            self.accs = [self.psp.tile([O, hn, S], F32, name=f"acc{ds}_{ci}",
                                       tag=f"acc{ci}")
                         for ci, (hb, hn) in enumerate(HCHUNKS)]
            self.first_mm = [True] * len(HCHUNKS)
        accs, first_mm = self.accs, self.first_mm
        have_next = si + 1 < nstages
        next_zetas = {}
        for c in range(C):
            xr = pool.tile([NDELTA, HP, WP], BF16,
                           name=f"xr{si}_{c}", tag="xr", bufs=3)
            xrf = xr.rearrange("p h w -> p (h w)")
            for a5 in range(5):
                src = _win_ap(
                    self.xbf_d[c:c + 1, :],
                    c * NPAD + (dpad - 3 + kd + a5) * HWP - 2 * WP - 2,
                    [(WP, 5), (1, 5), (1, HWP)])
                eng = nc.gpsimd if a5 >= 3 else nc.sync
                eng.dma_start(xrf[25 * a5:25 * a5 + 25, :], src)
            for ki in range(9):
                k = 9 * kd + ki
                kh, kw = (k // 3) % 3, k % 3
                win = xr[:, 3 + kh:3 + kh + S, 3 + kw:3 + kw + S]
                p = pool.tile([NDELTA, S, S], BF16,
                              name=f"p{si}_{c}_{k}", tag="ptile", bufs=3)
                nc.vector.tensor_tensor(p[:], zetas[ki][:], win, ALU.mult)
                wsl = self.wdc_s[0:NDELTA,
                                 (k * C + c) * O:(k * C + c + 1) * O]
                fin = (g == 2) and (c == C - 1) and (ki == 8)
                for ci, (hb, hn) in enumerate(HCHUNKS):
                    nc.tensor.matmul(accs[ci][:], wsl, p[:, hb:hb + hn, :],
                                     start=first_mm[ci], stop=fin)
                    first_mm[ci] = False
                # tags of next-stage taps j>=4 free up as tap j-4 finishes
                # its last read (c == C-1); emit the build right after
                if have_next and c == C - 1 and ki <= 4:
                    self.emit_zeta(si + 1, ki + 4, next_zetas)
            # next slice's off-conv spread over c positions (PE chunks
            # first, DVE evicts trail so the DVE FIFO never stalls),
            # then next stage's first zeta builds (tags z for j<=3
            # don't collide with this stage's)
            if have_next and g == 2:
                if c == 6:
                    self.emit_offconv_load(ds + 1)
                elif 8 <= c <= 16 and c % 2 == 0:
                    self.emit_offconv_chunk(ds + 1, (c - 8) // 2)
                elif c == 18:
                    self.emit_offconv_finish(ds + 1)
            if have_next and c in (20, 22, 24, 26):
                self.emit_zeta(si + 1, (c - 20) // 2, next_zetas)
        # ---------------- evict ----------------
        if g == 2:
            outp = pool.tile([O, S, S], F32, name=f"outp{ds}", tag="outp")
            for ci, (hb, hn) in enumerate(HCHUNKS):
                nc.scalar.activation(outp[:, hb:hb + hn, :], accs[ci][:],
                                     ACTF.Identity, bias=self.bdc_s[:, :],
                                     scale=1.0)
            nc.scalar.dma_start(self.out_d[:, ds * S * S:(ds + 1) * S * S],
                                outp.rearrange("p h w -> p (h w)"))
        return next_zetas


def build_kernel(nc: bass.Bass):
    x3_d = nc.dram_tensor("x3", [96, NPAD], BF16, kind="ExternalInput")
    xbf_d = nc.dram_tensor("x_bf", [C, NPAD], BF16, kind="ExternalInput")
    woff_d = nc.dram_tensor("w_off9", [96, 9 * 81], BF16, kind="ExternalInput")
    wdc_d = nc.dram_tensor("wdc_rep", [128, KV * C * O], BF16,
                           kind="ExternalInput")
    boff_d = nc.dram_tensor("b_off", [81, 1], F32, kind="ExternalInput")
    bdc_d = nc.dram_tensor("b_dc", [32, 1], F32, kind="ExternalInput")
    dv15_d = nc.dram_tensor("dvec15", [15, 1], F32, kind="ExternalInput")
    eps_d = nc.dram_tensor("epsv", [15, 1], F32, kind="ExternalInput")
    sel_d = nc.dram_tensor("seltap", [96, 27 * 16], BF16,
                           kind="ExternalInput")
    logsel_d = nc.dram_tensor("logsel", [15, 128], BF16, kind="ExternalInput")
    out_d = nc.dram_tensor("out", [O, NS_LOOP * S * S], F32,
                           kind="ExternalOutput")

    with TileContext(nc) as tc:
        with tc.tile_pool(name="fixed", bufs=1) as fixed, \
             tc.tile_pool(name="work", bufs=1) as pool, \
             tc.tile_pool(name="psum", bufs=1, space="PSUM") as psp:
            woff_s = fixed.tile([96, 9 * 81], BF16)
            nc.sync.dma_start(woff_s[:, :], woff_d[:, :])
            wdc_s = fixed.tile([128, KV * C * O], BF16)
            nc.sync.dma_start(wdc_s[:, :], wdc_d[:, :])
            boff_s = fixed.tile([81, 1], F32)
            nc.sync.dma_start(boff_s[:, :], boff_d[:, :])
            bdc_s = fixed.tile([32, 1], F32)
            nc.sync.dma_start(bdc_s[:, :], bdc_d[:, :])
            sel_s = fixed.tile([96, 27 * 16], BF16)
            nc.sync.dma_start(sel_s[:, :], sel_d[:, :])
            logsel_s = fixed.tile([15, 128], BF16)
            nc.sync.dma_start(logsel_s[:, :], logsel_d[:, :])
            dvec15 = fixed.tile([15, 1], F32)
            nc.sync.dma_start(dvec15[:, :], dv15_d[:, :])
            epsv = fixed.tile([15, 1], F32)
            nc.sync.dma_start(epsv[:, :], eps_d[:, :])

            # warm fixed tiles on DVE once so later DVE instructions don't
            # each carry a DMA-sem wait (HW wait-slot limit)
            warm = fixed.tile([1, 8], F32)
            for wsrc in [boff_s, bdc_s, sel_s, logsel_s, dvec15, epsv]:
                nc.vector.tensor_copy(warm[0:1, 0:1], wsrc[0:1, 0:1])

            em = _Emitter(nc, pool, psp, x3_d, xbf_d, out_d, woff_s, wdc_s,
                          boff_s, bdc_s, dvec15, epsv, sel_s, logsel_s)
            nstages = 3 * NS_LOOP
            em.emit_offconv(0)
            zetas = {}
            for j in range(9):
                em.emit_zeta(0, j, zetas)
            for si in range(nstages):
                zetas = em.emit_stage(si, zetas, nstages)
    return nc


# ---------------------------------------------------------------- entry
def kernel(x, w_off, b_off, w_dc, b_dc):
    x = np.asarray(x, np.float32)
    w_off = np.asarray(w_off, np.float32)
    b_off = np.asarray(b_off, np.float32)
    w_dc = np.asarray(w_dc, np.float32)
    b_dc = np.asarray(b_dc, np.float32)

    in_maps = [_build_core_inputs(x, w_off, b_off, w_dc, b_dc,
                                  core // 4, core % 4) for core in range(8)]

    nc = bacc.Bacc("TRN2", target_bir_lowering=False, debug=False,
                   enable_asserts=False, num_devices=8)
    build_kernel(nc)
    if not nc.is_finalized():
        nc.finalize()

    global LAST_RESULTS
    LAST_RESULTS = run_bass_kernel_spmd(nc, in_maps, list(range(8)))
    res = LAST_RESULTS.results

    out = np.zeros((B, O, S, S, S), np.float32)
    for core in range(8):
        b, q = core // 4, core % 4
        out[b, :, DP * q:DP * q + NS_LOOP] = \
            res[core]["out"].reshape(O, NS_LOOP, S, S).astype(np.float32)
    return out
